# revision 8
# baseline (speedup 1.0000x reference)
# Multi-head attention kernel for Trainium2 (8 NeuronCores, SPMD).
#
# Problem (hardcoded): X[4, 2048, 1024], W_k/W_q/W_v/W_u[1024, 1024], b_u[1024]
#   K = (X @ W_k.T) * s ; Q = (X @ W_q.T) * s ; V = (X @ W_v.T) * s   (s = 1024**-0.25)
#   S = Q @ K.T per head (16 heads, head_dim 64); P = softmax(S); Y = P @ V
#   out = Y @ W_u.T + b_u
#
# Sharding: core c handles (batch c//2, query-half c%2). Each core computes
# K/V projections for its batch's full sequence (needed by every query) and Q
# for its query half; outputs are disjoint [1024, 1024] slices of the result,
# so the host-side unshard is a pure concatenation.
#
# Per-core data layout (everything oriented so the PE contracts on partitions):
#   X^T  [e, t]    from host (layout-only transform during sharding)
#   K^T  [e', t]   feature-major; head h lives on partitions (h%2)*64.. of tile h//2
#   Q^T  [e', q]   same
#   V    [t, h, 65] token-major, 65th column = ones (gives softmax denominator
#                   for free as row 64 of the P@V matmul output)
#   S_T  [tk, q]   scores transposed; exp is layout-agnostic and the AV matmul
#                   wants P with tk on partitions, so softmax needs no transposes
#   Y^T  [e, q]    AV output, normalized by 1/D broadcast (DMA bounce via DRAM)
#   out  [q, e']   token-major final projection (lhsT = Y^T tiles)

import numpy as np

import concourse.bacc as bacc
import concourse.mybir as mybir
import concourse.tile as tile
from concourse.bass_utils import run_bass_kernel_spmd

FP32 = mybir.dt.float32
BF16 = mybir.dt.bfloat16
AF = mybir.ActivationFunctionType

P = 128
E = 1024          # embedding dim
H = 16            # heads
S = 64            # head dim
ET = E // P       # 8 contraction tiles over e
SCALE = float(1024.0 ** -0.25)

N_CORES = 8


def build_nc(T, TQ):
    """Build + compile the per-core Bass module. T = full seq len on this core,
    TQ = query rows handled by this core."""
    assert T % P == 0 and TQ % P == 0 and E == H * S
    TT = T // P   # key tiles

    nc = bacc.Bacc("TRN2", target_bir_lowering=False, debug=False,
                   enable_asserts=False)

    # xt arrives rotated so that this core's TQ query tokens are columns
    # 0..TQ-1 (attention is permutation-invariant over the key/token axis,
    # so K/V built from the rotated order give identical query outputs)
    xt = nc.dram_tensor("xt", [E, T], FP32, kind="ExternalInput").ap()
    wkt = nc.dram_tensor("wkt", [E, E], FP32, kind="ExternalInput").ap()
    wqt = nc.dram_tensor("wqt", [E, E], FP32, kind="ExternalInput").ap()
    wvt = nc.dram_tensor("wvt", [E, E], FP32, kind="ExternalInput").ap()
    wut = nc.dram_tensor("wut", [E, E], FP32, kind="ExternalInput").ap()
    bu = nc.dram_tensor("bu", [1, E], FP32, kind="ExternalInput").ap()
    out = nc.dram_tensor("out", [TQ, E], FP32, kind="ExternalOutput").ap()

    with tile.TileContext(nc) as tc:
        _build_kernel(tc, nc, T, TQ, TT, xt, wkt, wqt, wvt, wut, bu, out)
    nc.compile()
    return nc


def _chunks(total, step):
    return [(o, min(step, total - o)) for o in range(0, total, step)]


def _build_kernel(tc, nc, T, TQ, TT, xt, wkt, wqt, wvt, wut, bu, out):
    with (
        tc.tile_pool(name="persist", bufs=1) as persist,
        tc.tile_pool(name="psum", bufs=1, space="PSUM") as pspool,
        tc.tile_pool(name="dram", bufs=1, space="DRAM") as drampool,
    ):
        kt = persist.tile([P, ET, T], BF16, tag="kt", name="kt")
        qt = persist.tile([P, ET, TQ], BF16, tag="qt", name="qt")
        vv = persist.tile([P, TT, H, S + 1], BF16, tag="vv", name="vv")
        yt = persist.tile([P, ET, TQ], BF16, tag="yt", name="yt")
        bub = persist.tile([P, E], FP32, tag="bub", name="bub")

        nc.sync.dma_start(bub[:], bu.to_broadcast([P, E]))

        # ---------------- phase 1: projections ----------------
        with tc.tile_pool(name="ph1", bufs=1) as ph1:
            xtb = ph1.tile([P, ET, T], BF16, tag="xtb", name="xtb")
            for k in range(ET):
                xs = ph1.tile([P, T], FP32, tag="xs", bufs=2, name=f"xs{k}")
                nc.sync.dma_start(xs[:], xt[k * P:(k + 1) * P, :])
                nc.vector.tensor_copy(out=xtb[:, k, :], in_=xs[:])

            def load_w(wap, scaled, wtag, uname):
                wb = ph1.tile([P, ET, E], BF16, tag=wtag, bufs=1, name=uname)
                for k in range(ET):
                    ws = ph1.tile([P, E], FP32, tag="ws", bufs=2,
                                  name=f"ws_{uname}{k}")
                    nc.sync.dma_start(ws[:], wap[k * P:(k + 1) * P, :])
                    if scaled:
                        nc.vector.tensor_scalar_mul(wb[:, k, :], ws[:], SCALE)
                    else:
                        nc.vector.tensor_copy(out=wb[:, k, :], in_=ws[:])
                return wb

            # K projection -> kt[e', t] (feature-major)
            wbk = load_w(wkt, True, "wb0", "wbk")
            for m in range(ET):
                for t0, tw in _chunks(T, 1024):
                    ps = pspool.tile([P, 1024], FP32, tag="ps", bufs=2,
                                     name=f"psk{m}_{t0}")
                    for n0, nw in _chunks(tw, 512):
                        for k in range(ET):
                            nc.tensor.matmul(
                                ps[:, n0:n0 + nw],
                                lhsT=wbk[:, k, m * P:(m + 1) * P],
                                rhs=xtb[:, k, t0 + n0:t0 + n0 + nw],
                                start=(k == 0), stop=(k == ET - 1))
                    nc.scalar.copy(out=kt[:, m, t0:t0 + tw], in_=ps[:, :tw])

            # Q projection -> qt[e', q]
            wbq = load_w(wqt, True, "wb1", "wbq")
            for m in range(ET):
                for t0, tw in _chunks(TQ, 1024):
                    ps = pspool.tile([P, 1024], FP32, tag="ps", bufs=2,
                                     name=f"psq{m}_{t0}")
                    for n0, nw in _chunks(tw, 512):
                        for k in range(ET):
                            nc.tensor.matmul(
                                ps[:, n0:n0 + nw],
                                lhsT=wbq[:, k, m * P:(m + 1) * P],
                                rhs=xtb[:, k, t0 + n0:t0 + n0 + nw],
                                start=(k == 0), stop=(k == ET - 1))
                    nc.scalar.copy(out=qt[:, m, t0:t0 + tw], in_=ps[:, :tw])

            # V projection -> vv[t, h, 0:64] (token-major) + ones column
            wbv = load_w(wvt, True, "wb0", "wbv")
            for mt in range(TT):
                ps = pspool.tile([P, 1024], FP32, tag="ps", bufs=2,
                                 name=f"psv{mt}")
                for n0, nw in _chunks(E, 512):
                    for k in range(ET):
                        nc.tensor.matmul(
                            ps[:, n0:n0 + nw],
                            lhsT=xtb[:, k, mt * P:(mt + 1) * P],
                            rhs=wbv[:, k, n0:n0 + nw],
                            start=(k == 0), stop=(k == ET - 1))
                nc.scalar.copy(out=vv[:, mt, :, 0:S],
                               in_=ps[:].rearrange("p (h s) -> p h s", s=S))
                nc.vector.memset(vv[:, mt, :, S:S + 1], 1.0)

        # ---------------- phase 2: attention ----------------
        with tc.tile_pool(name="ph2", bufs=1) as ph2:
            for j in range(H // 2):
                ptiles = [
                    ph2.tile([P, TT, TQ], BF16, tag="pp", bufs=3,
                             name=f"p{2 * j + par}")
                    for par in range(2)
                ]
                # scores (transposed) + exp, both heads of the pair packed
                # into disjoint PE row-groups (K=64 each)
                for i in range(TT):
                    for par in range(2):
                        lo = par * S
                        ps = pspool.tile([P, TQ], FP32, tag="ps", bufs=2,
                                         name=f"s{j}_{i}_{par}")
                        for c0, cw in _chunks(TQ, 512):
                            nc.tensor.matmul(
                                ps[:, c0:c0 + cw],
                                lhsT=kt[lo:lo + S, j, i * P:(i + 1) * P],
                                rhs=qt[lo:lo + S, j, c0:c0 + cw],
                                start=True, stop=True)
                        nc.scalar.activation(ptiles[par][:, i, :], ps[:], AF.Exp)
                # P @ V_aug (row 64 of output = softmax denominator D)
                for par in range(2):
                    h = 2 * j + par
                    for c0, cw in _chunks(TQ, 512):
                        av = pspool.tile([P, 512], FP32, tag="av", bufs=4,
                                         name=f"av{h}_{c0}")
                        for i in range(TT):
                            nc.tensor.matmul(
                                av[0:S + 1, :cw],
                                lhsT=vv[:, i, h, :],
                                rhs=ptiles[par][:, i, c0:c0 + cw],
                                start=(i == 0), stop=(i == TT - 1))
                        # 1/D on the D row's own lane, bounce through DRAM to
                        # broadcast across partitions 0..63
                        dr = ph2.tile([S + 1, 512], FP32, tag="dr", bufs=2,
                                      name=f"dr{h}_{c0}")
                        nc.vector.reciprocal(
                            out=dr[S:S + 1, :cw], in_=av[S:S + 1, :cw])
                        db = drampool.tile([1, 512], FP32, tag="db", bufs=8,
                                           name=f"db{h}_{c0}")
                        nc.sync.dma_start(db[:, :cw], dr[S:S + 1, :cw])
                        rbc = ph2.tile([S, 512], FP32, tag="rbc", bufs=2,
                                       name=f"rbc{h}_{c0}")
                        nc.sync.dma_start(rbc[:, :cw],
                                          db[:, :cw].to_broadcast([S, cw]))
                        if par == 0:
                            nc.vector.tensor_mul(
                                out=yt[0:S, j, c0:c0 + cw],
                                in0=av[0:S, :cw], in1=rbc[:, :cw])
                        else:
                            # odd head's Y lives on partitions 64..127 of the
                            # yt tile; engines are lane-locked, so normalize on
                            # lanes 0..63 then DMA-shift across partitions
                            tmp = ph2.tile([S, 512], BF16, tag="tmp", bufs=2,
                                           name=f"tmp{h}_{c0}")
                            nc.vector.tensor_mul(out=tmp[:, :cw],
                                                 in0=av[0:S, :cw],
                                                 in1=rbc[:, :cw])
                            nc.sync.dma_start(yt[S:P, j, c0:c0 + cw],
                                              tmp[:, :cw])

        # ---------------- phase 3: output projection ----------------
        with tc.tile_pool(name="ph3", bufs=1) as ph3:
            wub = ph3.tile([P, ET, E], BF16, tag="wub", name="wub")
            for k in range(ET):
                ws = ph3.tile([P, E], FP32, tag="wus", bufs=2, name=f"wus{k}")
                nc.sync.dma_start(ws[:], wut[k * P:(k + 1) * P, :])
                nc.vector.tensor_copy(out=wub[:, k, :], in_=ws[:])
            for m in range(TQ // P):
                for n0, nw in _chunks(E, 512):
                    ps = pspool.tile([P, 512], FP32, tag="av", bufs=4,
                                     name=f"o{m}_{n0}")
                    for k in range(ET):
                        nc.tensor.matmul(
                            ps[:, :nw],
                            lhsT=yt[:, k, m * P:(m + 1) * P],
                            rhs=wub[:, k, n0:n0 + nw],
                            start=(k == 0), stop=(k == ET - 1))
                    ot = ph3.tile([P, 512], FP32, tag="ot", bufs=3,
                                  name=f"ot{m}_{n0}")
                    nc.vector.tensor_add(out=ot[:, :nw], in0=ps[:, :nw],
                                         in1=bub[:, n0:n0 + nw])
                    nc.sync.dma_start(out[m * P:(m + 1) * P, n0:n0 + nw],
                                      ot[:, :nw])


_NC_CACHE = {}


def _get_nc(T, TQ):
    key = (T, TQ)
    if key not in _NC_CACHE:
        _NC_CACHE[key] = build_nc(T, TQ)
    return _NC_CACHE[key]


def make_in_maps(X, W_k, W_q, W_v, W_u, b_u):
    X = np.asarray(X, np.float32)
    b, t, e = X.shape
    tq = t // 2
    wk_t = np.ascontiguousarray(np.asarray(W_k, np.float32).T)
    wq_t = np.ascontiguousarray(np.asarray(W_q, np.float32).T)
    wv_t = np.ascontiguousarray(np.asarray(W_v, np.float32).T)
    wu_t = np.ascontiguousarray(np.asarray(W_u, np.float32).T)
    bu2 = np.ascontiguousarray(np.asarray(b_u, np.float32).reshape(1, e))
    in_maps = []
    for c in range(N_CORES):
        bi, qo = c // 2, (c % 2) * tq
        xt_np = X[bi].T
        # rotate so this core's query tokens are columns 0..tq-1
        xt_np = np.ascontiguousarray(
            np.concatenate([xt_np[:, qo:], xt_np[:, :qo]], axis=1))
        in_maps.append({
            "xt": xt_np,
            "wkt": wk_t, "wqt": wq_t, "wvt": wv_t, "wut": wu_t,
            "bu": bu2,
        })
    return in_maps


def run(inputs, trace=False, **kwargs):
    """Run on hardware; returns (full output, BassKernelResults)."""
    X = np.asarray(inputs["X"], np.float32)
    b, t, e = X.shape
    tq = t // 2
    nc = _get_nc(t, tq)
    in_maps = make_in_maps(X, inputs["W_k"], inputs["W_q"], inputs["W_v"],
                           inputs["W_u"], inputs["b_u"])
    res = run_bass_kernel_spmd(nc, in_maps, core_ids=list(range(N_CORES)),
                               trace=trace, **kwargs)
    full = np.empty((b, t, e), np.float32)
    for c in range(N_CORES):
        full[c // 2, (c % 2) * tq:(c % 2) * tq + tq, :] = res.results[c]["out"]
    return full, res


def kernel(**inputs):
    full, _ = run(inputs)
    return full
